# revision 1
# baseline (speedup 1.0000x reference)
"""Two-layer GRU (16->128->128) + FC(128->24) head on 8 Trainium2 NeuronCores.

Strategy: data-parallel over the batch (4096 -> 512 per core); tiny weights
replicated. On each core the hidden state lives transposed in SBUF as
[H=128 partitions, B=512 free]. Per time step, gate pre-activations are
accumulated in PSUM by matmuls (input-projection + recurrent + biases
folded in), sigmoid/tanh run on the scalar engine, and the cell update is
spread across vector + gpsimd engines.

Execution path: one cached jit(shard_map) callable whose body is ONLY the
bass_exec custom call; the host repacks x to the time-major transposed
[L, 17, BL] f16 layout, device_puts pre-sharded arguments, and gathers the
small [24, 512]-per-core output. The two GRU layers are software-pipelined
(layer 2 one step behind layer 1) so the per-step serial chain is a single
layer's, and the h-update uses h' = (1-z)*n + z*h with the (1-z) and z*h
pieces precomputed off the critical path.

Self-contained: hardcodes all shapes; no file I/O.
"""

import numpy as np

import bass_rust
import concourse.bass as bass
import concourse.mybir as mybir
from concourse.tile import TileContext
from concourse.vector_clock import ScopedClock

N_CORES = 8
B_TOT = 4096
L = 128          # sequence length (= 2*1024/16)
D = 16           # per-step input features
DA = 17          # + ones row (bias folding for layer 1)
H = 128          # hidden
G3 = 3 * H       # 384 stacked gates (r, z, n)
BL = B_TOT // N_CORES  # 512 batch per core
NCLS = 24
CHUNK = 8        # time steps of x staged into SBUF per DMA

F32 = mybir.dt.float32
F32R = mybir.dt.float32r
F16 = mybir.dt.float16
BF16 = mybir.dt.bfloat16
AF = mybir.ActivationFunctionType
OP = mybir.AluOpType

# Tunables (grid-searched via TimelineSim, validated on HW).
CONFIG = {
    "dtype": "f16",      # gate/h/weight/x dtype: "f32r" | "f16" | "bf16"
    "zh1": True,         # L1 h-update via oz/zh trick (2 post-tanh hops)
    "zh2": True,         # L2 h-update likewise (else d/e/h chain)
    "mm_order": "rz_first",  # prz matmul order: "rz_first" | "r_first"
    "t22_late": False,   # emit layer-2 t2/imm after layer-1's h update
    "t2_early": False,   # emit layer-1 t2 right after its sigmoids
    "prep_late": False,  # emit oz/zh after t2/imm instead of after sig
    "prep1_eng": "v",    # engine for layer-1 oz/zh
    "prep2_eng": "v",    # engine for layer-2 oz/zh
    "t2_eng": "v",       # engine for t2 = (pn + b) * r
    "upd1_eng": "v",     # engine for layer-1 u / h'
    "oz1_eng": "v",      # engine for layer-1 oz = 1 - z
    "oz2_eng": "v",      # engine for layer-2 oz = 1 - z
    "upd2_eng": "v",     # engine for layer-2 u / h' 
    "rz1_fused": False,  # one sigmoid over [H, 2BL] for layer-1 r|z
    "rz2_fused": False,  # (needs PSUM bias; only valid when rz_bias is None)
    "h_bufs": 2,
    "work_bufs": 3,
}

_DT = {"f32r": F32R, "f16": F16, "bf16": BF16}
_NP_DT = {"f32r": np.float32, "f16": np.float16}


class SplitDrainTileContext(TileContext):
    """Walrus (CoreV3) rejects instructions carrying >2 sync waits; Tile's
    kernel-tail drain accumulates one wait per outstanding engine/DMA-queue
    sem. Split them across a chain of drains (1 wait each)."""

    def _drain_and_barrier(self, tick_clock, wait_clock):
        nc = self.nc
        drain_inst = nc.sync.drain()
        wait_clock.add_sem_waits(
            drain_inst.ins, ScopedClock({None: tick_clock.global_clock})
        )
        si = drain_inst.ins.sync_info
        if si is not None and len(si.on_wait) > 1:
            waits = list(si.on_wait)
            si.on_wait = waits[:1]
            for w in waits[1:]:
                d2 = nc.sync.drain()
                d2.ins.sync_info = bass_rust.SyncInfo(on_wait=[w], on_update=[])
        nc.all_engine_barrier()
        popped = nc._tile_sem_poison_stack.pop()
        assert popped is self._sem_poison
        nc.clear_and_free_semaphores(list(self.sems.allocated().values()))
        nc.all_engine_barrier()


def _split_excess_waits(nc: bass.Bass, max_waits: int = 1) -> None:
    """Walrus (CoreV3 setupSyncWait) accepts at most 2 sem waits per
    instruction; Tile occasionally attaches 3+. Hoist the excess onto
    EventSemaphore instructions inserted right before the offender on the
    same engine (serial waits AND together)."""
    n = 0
    for fn in nc.m.functions:
        for bb in fn.blocks:
            out = []
            dirty = False
            for inst in bb.instructions:
                si = inst.sync_info
                if si is not None and len(si.on_wait) > max_waits:
                    waits = list(si.on_wait)
                    extra = waits[: len(waits) - max_waits]
                    for w in extra:
                        ev = mybir.InstEventSemaphore(
                            name=f"evs-waitsplit-{n}", ins=[], outs=[]
                        )
                        n += 1
                        ev.engine = inst.engine
                        ev.sync_info = bass_rust.SyncInfo(
                            on_wait=[w], on_update=[]
                        )
                        out.append(ev)
                    si.on_wait = waits[len(waits) - max_waits :]
                    dirty = True
                out.append(inst)
            if dirty:
                bb.instructions = out


def build_program(for_sim: bool = False, n_steps: int = L,
                  repeats: int = 1) -> bass.Bass:
    # for_sim: skip the walrus wait-limit workarounds (post-hoc IR mutations
    # that CoreSim's bookkeeping doesn't understand); semantics identical.
    # repeats: run the whole forward pass R times back-to-back (timing
    # programs; the marginal wall-clock per extra pass is the HW exec time).
    nc = bass.Bass()

    DT = _DT[CONFIG["dtype"]]
    xT_d = nc.declare_dram_parameter("xT", [L, DA, BL], DT, isOutput=False)
    l1w_d = nc.declare_dram_parameter("l1w", [DA, G3], DT, isOutput=False)
    hh1_d = nc.declare_dram_parameter("hh1w", [H, G3], DT, isOutput=False)
    ih2_d = nc.declare_dram_parameter("ih2w", [H, G3], DT, isOutput=False)
    hh2_d = nc.declare_dram_parameter("hh2w", [H, G3], DT, isOutput=False)
    bias_d = nc.declare_dram_parameter("bvec", [H, 5], F32, isOutput=False)
    fcw_d = nc.declare_dram_parameter("fcw", [H, NCLS], DT, isOutput=False)
    fcb_d = nc.declare_dram_parameter("fcb", [NCLS, 1], F32, isOutput=False)
    ident_d = nc.declare_dram_parameter("ident", [H, H], DT, isOutput=False)
    out_d = nc.declare_dram_parameter("outT", [NCLS, BL], F32, isOutput=True)

    tc_cls = TileContext if for_sim else SplitDrainTileContext
    with tc_cls(nc) as tc:
        with (
            tc.tile_pool(name="singles", bufs=1) as singles,
            tc.tile_pool(name="xchunks", bufs=3) as xpool,
            tc.tile_pool(name="hstate", bufs=CONFIG["h_bufs"]) as hpool,
            tc.tile_pool(name="work", bufs=CONFIG["work_bufs"]) as work,
            tc.tile_pool(name="prz", bufs=1, space="PSUM") as przpool,
            tc.tile_pool(name="pnx", bufs=1, space="PSUM") as pnxpool,
        ):
            # --- constant loads -------------------------------------------
            l1w = singles.tile([DA, G3], DT, tag="l1w")
            hh1w = singles.tile([H, G3], DT, tag="hh1w")
            ih2w = singles.tile([H, G3], DT, tag="ih2w")
            hh2w = singles.tile([H, G3], DT, tag="hh2w")
            sbias = singles.tile([H, 5], F32, tag="sbias")
            fcw = singles.tile([H, NCLS], DT, tag="fcw")
            fcb = singles.tile([NCLS, 1], F32, tag="fcb")
            ident = singles.tile([H, H], DT, tag="ident")
            nc.sync.dma_start(out=ident[:], in_=ident_d[:])
            nc.sync.dma_start(out=l1w[:], in_=l1w_d[:])
            nc.sync.dma_start(out=hh1w[:], in_=hh1_d[:])
            nc.sync.dma_start(out=ih2w[:], in_=ih2_d[:])
            nc.sync.dma_start(out=hh2w[:], in_=hh2_d[:])
            nc.sync.dma_start(out=sbias[:], in_=bias_d[:])
            nc.sync.dma_start(out=fcw[:], in_=fcw_d[:])
            nc.sync.dma_start(out=fcb[:], in_=fcb_d[:])

            ENG = {"v": nc.vector, "g": nc.gpsimd}

            class Cell:
                """Stage-split GRU cell so the two layers' instruction
                streams can be interleaved (software pipelining: layer 2
                runs one time step behind layer 1). Engines execute their
                streams in order, so emission order determines the
                schedule."""

                def __init__(self, tag, xw, hw, rz_bias, n_hh_bias,
                             n_ih_bias, use_zh):
                    self.tag = tag
                    self.xw = xw            # [K, G3] lhsT for the input proj
                    self.hw = hw            # [H, G3] recurrent lhsT
                    self.rz_bias = rz_bias  # None (folded) or (r_ap, z_ap)
                    self.n_hh_bias = n_hh_bias
                    self.n_ih_bias = n_ih_bias
                    self.use_zh = use_zh

                def in_mms(self, x_rhs, alone=False):
                    # Input-projection matmuls (only need x / h1): allocate
                    # the PSUM tiles and write the start-of-accumulation
                    # parts. Emitted an iteration EARLY so they don't sit on
                    # the h -> gates critical path.
                    tag = self.tag
                    self.prz = przpool.tile([H, 2 * BL], F32, tag=f"prz{tag}")
                    self.px = pnxpool.tile([H, BL], F32, tag=f"px{tag}")
                    nc.tensor.matmul(self.prz[:, 0:BL], self.xw[:, 0:H],
                                     x_rhs, start=True, stop=alone)
                    nc.tensor.matmul(self.prz[:, BL:], self.xw[:, H : 2 * H],
                                     x_rhs, start=True, stop=alone)
                    nc.tensor.matmul(self.px[:], self.xw[:, 2 * H :], x_rhs,
                                     start=True, stop=False)

                def rec_mms(self, h_prev):
                    # Recurrent matmuls: the only PE work on the h critical
                    # path. r first (it gates the sigmoid -> t2 chain).
                    tag = self.tag
                    self.pn = pnxpool.tile([H, BL], F32, tag=f"pn{tag}")
                    nc.tensor.matmul(self.prz[:, 0:BL], self.hw[:, 0:H],
                                     h_prev[:], start=False, stop=True)
                    nc.tensor.matmul(self.prz[:, BL:], self.hw[:, H : 2 * H],
                                     h_prev[:], start=False, stop=True)
                    nc.tensor.matmul(self.pn[:], self.hw[:, 2 * H :],
                                     h_prev[:], start=True, stop=True)

                def full_mms(self, x_rhs, h_prev):
                    # Input + recurrent together (layer 2: the "input" is
                    # h1 from this very iteration, so no early emission).
                    tag = self.tag
                    self.prz = przpool.tile([H, 2 * BL], F32, tag=f"prz{tag}")
                    self.px = pnxpool.tile([H, BL], F32, tag=f"px{tag}")
                    prz = self.prz
                    nc.tensor.matmul(prz[:, 0:BL], self.xw[:, 0:H], x_rhs,
                                     start=True, stop=h_prev is None)
                    if h_prev is not None:
                        nc.tensor.matmul(prz[:, 0:BL], self.hw[:, 0:H],
                                         h_prev[:], start=False, stop=True)
                    nc.tensor.matmul(prz[:, BL:], self.xw[:, H : 2 * H],
                                     x_rhs, start=True, stop=h_prev is None)
                    if h_prev is not None:
                        nc.tensor.matmul(prz[:, BL:], self.hw[:, H : 2 * H],
                                         h_prev[:], start=False, stop=True)
                        self.pn = pnxpool.tile([H, BL], F32, tag=f"pn{tag}")
                        nc.tensor.matmul(self.pn[:], self.hw[:, 2 * H :],
                                         h_prev[:], start=True, stop=True)
                    nc.tensor.matmul(self.px[:], self.xw[:, 2 * H :], x_rhs,
                                     start=True, stop=False)

                def sig(self):
                    tag = self.tag
                    if self.rz_bias is None and CONFIG[f"rz{tag}_fused"]:
                        rz = work.tile([H, 2 * BL], DT, tag=f"rz{tag}")
                        nc.scalar.activation(rz[:], self.prz[:], AF.Sigmoid)
                        self.r, self.z = rz[:, 0:BL], rz[:, BL:]
                        return
                    rb = dict(bias=self.rz_bias[0]) if self.rz_bias else {}
                    zb = dict(bias=self.rz_bias[1]) if self.rz_bias else {}
                    rt = work.tile([H, BL], DT, tag=f"r{tag}")
                    nc.scalar.activation(rt[:], self.prz[:, 0:BL],
                                         AF.Sigmoid, **rb)
                    zt = work.tile([H, BL], DT, tag=f"z{tag}")
                    nc.scalar.activation(zt[:], self.prz[:, BL:],
                                         AF.Sigmoid, **zb)
                    self.r, self.z = rt[:], zt[:]

                def prep(self, h_prev):
                    # Off-critical-path pieces of the h update:
                    # oz = 1 - z, zh = z * h_prev.
                    if not self.use_zh:
                        return
                    tag = self.tag
                    eng = ENG[CONFIG[f"prep{tag}_eng"]]
                    oz_eng = ENG[CONFIG[f"oz{tag}_eng"]]
                    self.oz = work.tile([H, BL], DT, tag=f"oz{tag}")
                    oz_eng.tensor_scalar(self.oz[:], self.z, -1.0, 1.0,
                                         op0=OP.mult, op1=OP.add)
                    if h_prev is not None:
                        self.zh = work.tile([H, BL], DT, tag=f"zh{tag}")
                        eng.tensor_mul(self.zh[:], self.z, h_prev[:])

                def t2(self, h_prev):
                    tag = self.tag
                    self.t2t = work.tile([H, BL], DT, tag=f"t2{tag}")
                    if h_prev is not None:
                        # t2 = (hn + b_hh_n) * r
                        ENG[CONFIG['t2_eng']].scalar_tensor_tensor(
                            self.t2t[:], self.pn[:], self.n_hh_bias, self.r,
                            op0=OP.add, op1=OP.mult)
                    else:
                        ENG[CONFIG['t2_eng']].tensor_scalar_mul(self.t2t[:], self.r,
                                                    self.n_hh_bias)

                def imm(self):
                    # px += I.T @ t2 on the PE, then tanh straight off PSUM
                    nc.tensor.matmul(self.px[:], ident[:], self.t2t[:],
                                     start=False, stop=True)

                def tanh(self):
                    tag = self.tag
                    nb = dict(bias=self.n_ih_bias) if self.n_ih_bias is not None else {}
                    self.n = work.tile([H, BL], DT, tag=f"n{tag}")
                    nc.scalar.activation(self.n[:], self.px[:], AF.Tanh, **nb)

                def update(self, h_prev):
                    tag = self.tag
                    eng = ENG[CONFIG[f"upd{tag}_eng"]]
                    h_new = hpool.tile([H, BL], DT, tag=f"h{tag}")
                    if self.use_zh:
                        # h' = n*(1-z) + z*h  (2 hops after tanh)
                        if h_prev is None:
                            eng.tensor_mul(h_new[:], self.n[:], self.oz[:])
                        else:
                            u = work.tile([H, BL], DT, tag=f"u{tag}")
                            eng.tensor_mul(u[:], self.n[:], self.oz[:])
                            eng.tensor_add(h_new[:], u[:], self.zh[:])
                    else:
                        # h' = n + z*(h - n)  (3 hops after tanh)
                        d = work.tile([H, BL], DT, tag=f"d{tag}")
                        if h_prev is not None:
                            nc.vector.tensor_sub(d[:], h_prev[:], self.n[:])
                        else:
                            nc.vector.tensor_scalar_mul(d[:], self.n[:], -1.0)
                        e = work.tile([H, BL], DT, tag=f"e{tag}")
                        nc.vector.tensor_mul(e[:], self.z, d[:])
                        nc.vector.tensor_add(h_new[:], self.n[:], e[:])
                    return h_new

            c1 = Cell("1", l1w, hh1w, None, sbias[:, 0:1], None,
                      CONFIG["zh1"])
            c2 = Cell("2", ih2w, hh2w, (sbias[:, 1:2], sbias[:, 2:3]),
                      sbias[:, 3:4], sbias[:, 4:5], CONFIG["zh2"])

            xc = None

            def xg(t):
                nonlocal xc
                if t % CHUNK == 0:
                    xc = xpool.tile([DA, CHUNK, BL], DT, tag="xc")
                    nc.sync.dma_start(
                        out=xc[:],
                        in_=xT_d[t : t + CHUNK].rearrange("t d b -> d t b"))
                return xc[:, t % CHUNK, :]

            for _rep in range(repeats):
              # Prologue: layer-1 step 0 alone.
              c1.in_mms(xg(0), alone=True)
              c1.sig()
              c1.prep(None)
              c1.t2(None)
              c1.imm()
              c1.tanh()
              h1_prev = c1.update(None)
              if n_steps > 1:
                c1.in_mms(xg(1))

              h2_prev = None
              for i in range(1, n_steps):
                  # Layer 1 works on step i while layer 2 works on step i-1.
                  c1.rec_mms(h1_prev)
                  c1.sig()
                  if CONFIG["t2_early"]:
                      c1.t2(h1_prev)
                  if not CONFIG["prep_late"]:
                      c1.prep(h1_prev)
                  c2.full_mms(h1_prev, h2_prev)
                  c2.sig()
                  if not CONFIG["prep_late"]:
                      c2.prep(h2_prev)
                  if not CONFIG["t2_early"]:
                      c1.t2(h1_prev)
                  c1.imm()
                  if CONFIG["prep_late"]:
                      c1.prep(h1_prev)
                  c1.tanh()
                  if not CONFIG["t22_late"]:
                      c2.t2(h2_prev)
                      c2.imm()
                      if CONFIG["prep_late"]:
                          c2.prep(h2_prev)
                  h1_new = c1.update(h1_prev)
                  if CONFIG["t22_late"]:
                      c2.t2(h2_prev)
                      c2.imm()
                      if CONFIG["prep_late"]:
                          c2.prep(h2_prev)
                  if i + 1 < n_steps:
                      c1.in_mms(xg(i + 1))
                  c2.tanh()
                  h2_prev = c2.update(h2_prev)
                  h1_prev = h1_new

              # Epilogue: layer-2 step L-1.
              c2.full_mms(h1_prev, h2_prev)
              c2.sig()
              c2.prep(h2_prev)
              c2.t2(h2_prev)
              c2.imm()
              c2.tanh()
              h2_prev = c2.update(h2_prev)

              # ---------------- FC head ------------------------------------
              pfc = pnxpool.tile([NCLS, BL], F32, tag="pn1")
              nc.tensor.matmul(pfc[:], fcw[:], h2_prev[:], start=True, stop=True)
              outs = work.tile([NCLS, BL], F32, tag="outs")
              nc.scalar.activation(outs[:], pfc[:], AF.Identity, bias=fcb[:])
              nc.sync.dma_start(out=out_d[:], in_=outs[:])

    if not for_sim:
        _split_excess_waits(nc)
    return nc


def prep_weights(inputs: dict) -> dict:
    """Pack the small GRU/FC weights into the kernel's layouts (host numpy)."""
    w_ih1 = np.asarray(inputs["w_ih1"], np.float32)
    w_hh1 = np.asarray(inputs["w_hh1"], np.float32)
    b_ih1 = np.asarray(inputs["b_ih1"], np.float32)
    b_hh1 = np.asarray(inputs["b_hh1"], np.float32)
    w_ih2 = np.asarray(inputs["w_ih2"], np.float32)
    w_hh2 = np.asarray(inputs["w_hh2"], np.float32)
    b_ih2 = np.asarray(inputs["b_ih2"], np.float32)
    b_hh2 = np.asarray(inputs["b_hh2"], np.float32)
    fc_w = np.asarray(inputs["fc_w"], np.float32)
    fc_b = np.asarray(inputs["fc_b"], np.float32)

    # layer-1 combined input-proj weights + bias row.
    # r/z columns carry b_ih1+b_hh1; n columns carry b_ih1 only (b_hh1_n must
    # be applied inside r*(hn+b_hh1_n)).
    l1w = np.empty((DA, G3), np.float32)
    l1w[0:D, :] = w_ih1.T
    bias_row = b_ih1.copy()
    bias_row[0 : 2 * H] += b_hh1[0 : 2 * H]
    l1w[D, :] = bias_row

    bvec = np.stack(
        [
            b_hh1[2 * H : 3 * H],                     # col 0: L1 n-gate hh bias
            (b_ih2 + b_hh2)[0:H],                     # col 1: L2 r bias
            (b_ih2 + b_hh2)[H : 2 * H],               # col 2: L2 z bias
            b_hh2[2 * H : 3 * H],                     # col 3: L2 n-gate hh bias
            b_ih2[2 * H : 3 * H],                     # col 4: L2 n-gate ih bias
        ],
        axis=1,
    ).astype(np.float32)

    if CONFIG["dtype"] == "bf16":
        import ml_dtypes
        ndt = np.dtype(ml_dtypes.bfloat16)
    else:
        ndt = _NP_DT[CONFIG["dtype"]]
    return {
        "l1w": np.ascontiguousarray(l1w).astype(ndt),
        "hh1w": np.ascontiguousarray(w_hh1.T).astype(ndt),
        "ih2w": np.ascontiguousarray(w_ih2.T).astype(ndt),
        "hh2w": np.ascontiguousarray(w_hh2.T).astype(ndt),
        "bvec": bvec,
        "fcw": np.ascontiguousarray(fc_w.T).astype(ndt),
        "fcb": np.ascontiguousarray(fc_b[:, None]),
        "ident": np.eye(H, dtype=np.float32).astype(ndt),
    }


_EXEC = {}


def get_executor(repeats: int = 1):
    """Build (once per `repeats`) the jitted shard_map callable around the
    bass program.

    The jit body contains ONLY the bass_exec custom call (the neuronx-cc
    hook rejects modules with extra computations), so all input repacking
    happens host-side and the argument arrays are staged on device by
    device_inputs(). repeats > 1 builds a timing variant that runs the
    whole forward pass that many times back-to-back on-device.
    """
    if repeats in _EXEC:
        return _EXEC[repeats]

    import jax
    from jax.experimental.shard_map import shard_map
    from jax.sharding import Mesh, PartitionSpec, NamedSharding
    from concourse import bass2jax

    bass2jax.install_neuronx_cc_hook()

    nc = build_program(repeats=repeats)
    partition_name = nc.partition_id_tensor.name if nc.partition_id_tensor else None
    in_names, out_names, out_avals = [], [], []
    for alloc in nc.m.functions[0].allocations:
        if not isinstance(alloc, mybir.MemoryLocationSet):
            continue
        name = alloc.memorylocations[0].name
        if alloc.kind == "ExternalInput":
            if name != partition_name:
                in_names.append(name)
        elif alloc.kind == "ExternalOutput":
            shape = tuple(alloc.tensor_shape)
            dtype = mybir.dt.np(alloc.dtype)
            out_names.append(name)
            out_avals.append(jax.core.ShapedArray(shape, dtype))
    all_in_names = list(in_names) + list(out_names)
    if partition_name is not None:
        all_in_names.append(partition_name)

    def _body(*args):
        operands = list(args)
        if partition_name is not None:
            operands.append(bass2jax.partition_id_tensor())
        outs = bass2jax._bass_exec_p.bind(
            *operands,
            out_avals=tuple(out_avals),
            in_names=tuple(all_in_names),
            out_names=tuple(out_names),
            lowering_input_output_aliases=(),
            sim_require_finite=True,
            sim_require_nnan=True,
            nc=nc,
        )
        return tuple(outs)

    devices = jax.devices()[:N_CORES]
    mesh = Mesh(np.asarray(devices), ("core",))
    spec = PartitionSpec("core")
    n_args = len(in_names) + len(out_avals)
    sharded = jax.jit(
        shard_map(
            _body,
            mesh=mesh,
            in_specs=(spec,) * n_args,
            out_specs=(spec,) * len(out_avals),
            check_rep=False,
        )
    )
    _EXEC[repeats] = {
        "fn": sharded,
        "mesh": mesh,
        "sharding": NamedSharding(mesh, spec),
        "in_names": in_names,
        "out_names": out_names,
        "out_avals": out_avals,
        "nc": nc,
    }
    return _EXEC[repeats]


def prep_host(inputs: dict) -> dict:
    """Host-side repack of the full inputs into per-core concatenated
    arrays keyed by DRAM tensor name (axis 0 = core for shard_map)."""
    x = np.asarray(inputs["x"])
    xr = x.astype(np.float16).reshape(N_CORES, BL, 2, L, D // 2)
    xt = xr.transpose(0, 3, 2, 4, 1).reshape(N_CORES, L, D, BL)
    xT = np.concatenate(
        [xt, np.ones((N_CORES, L, 1, BL), np.float16)], axis=2)

    w = prep_weights(inputs)
    arrs = {"xT": xT.reshape(N_CORES * L, DA, BL)}
    for name, val in w.items():
        arrs[name] = np.broadcast_to(
            val, (N_CORES,) + val.shape).reshape((N_CORES * val.shape[0],)
                                                + val.shape[1:])
    return arrs


def device_inputs(inputs: dict):
    """Host prep + H2D: the jit's argument list, already sharded on the
    mesh. Output operands are staged zero buffers (the kernel writes every
    element of outT, so they can be reused across calls; not donated)."""
    import jax

    ex = get_executor(1)
    arrs = prep_host(inputs)
    args = [arrs[n] for n in ex["in_names"]]
    for a in ex["out_avals"]:
        args.append(np.zeros((N_CORES * a.shape[0],) + tuple(a.shape[1:]),
                             a.dtype))
    return [jax.device_put(a, ex["sharding"]) for a in args]


def assemble_output(outs) -> np.ndarray:
    # outT concat over cores: (8*24, 512) -> (4096, 24)
    outT = np.asarray(outs[0])
    return np.ascontiguousarray(
        outT.reshape(N_CORES, NCLS, BL).transpose(0, 2, 1).reshape(B_TOT, NCLS)
    ).astype(np.float32)


def kernel(**inputs) -> np.ndarray:
    import jax

    ex = get_executor(1)
    args = device_inputs(inputs)
    outs = ex["fn"](*args)
    jax.block_until_ready(outs)
    return assemble_output(outs)

